# revision 31
# baseline (speedup 1.0000x reference)
"""Trainium2 Bass kernel for the 9-layer dense MLP (dropout-mask training forward).

Strategy (pure data parallel, 8 cores, 8192 batch rows each):
  - Activations kept transposed on-chip: features on partitions, batch cols on free dim.
    Each layer computes zT = W^T @ hT via nc.tensor.matmul(out, lhsT=W, rhs=hT).
  - fp16 weights/activations/masks (fp32 PSUM accumulation), fp32 biases + output.
  - Dropout masks binarized on host ({0,1} fp16); the 1/keep scale is folded into the
    next layer's weights.
  - Layer-major processing within blocks of 2048 batch columns (4 PSUM sub-tiles of
    512): per (c,k) weight tile, 4 consecutive matmuls share one LDWEIGHTS and
    pipeline back-to-back on the PE; PSUM drained per 512-chunk with fused bias+relu
    alternating ScalarE/VectorE; mask multiply per (layer, sub-tile) on DVE/GpSimd.
  - Layers 6/7/8 partition-packed (partition offsets 0/64/96 via matmul tile_position)
    sharing 2-bank PSUM tiles per 1024-column half and one packed mask chunk.
"""

import sys

sys.path.insert(0, "/opt/trn_rl_repo")

import numpy as np

DIMS = [256, 128, 256, 512, 256, 128, 64, 32, 16, 10]
NCORES = 8
BATCH = 65536
SHARD = BATCH // NCORES  # 8192
MSUB = 512               # PSUM sub-tile columns
BLK = 2048               # block columns
NBLK = SHARD // BLK      # 4
NSUB = BLK // MSUB       # 4

# pack chunk layout (each chunk = 128 partitions x 8192 cols, fp16):
#   0,1: xT        2: m1        3,4: m2      5-8: m3
#   9,10: m4       11: m5       12: m6/m7/m8 partition-packed at rows 0/64/96
NPACK = 13

_PROG = {}


def _raise_sbuf_cap():
    # tile_utils.max_sbuf_usage is a stale 192KB constant; cayman has 208KB usable.
    import concourse.tile_utils as tu

    if getattr(tu, "max_sbuf_usage", 0) < 206 * 1024:
        tu.max_sbuf_usage = 206 * 1024


def _dedup_ldweights(nc):
    """Remove back-to-back redundant LDWEIGHTS (same stationary operand) so
    consecutive same-weight matmuls pipeline on the PE. Only drops LDW
    instructions that carry no semaphore waits/updates."""
    removed = 0
    for fn in nc.m.functions:
        for blk in fn.blocks:
            il = blk.instructions
            keep, last_sig = [], None
            for inst in il:
                nm = type(inst).__name__
                if nm == "InstLdweights":
                    sig = (str(inst.ins[0]), str(inst.is_transpose), str(inst.perf_mode),
                           str(getattr(inst, "tile_position", None)))
                    si = inst.sync_info
                    clean = si is None or (not si.on_wait and not si.on_update)
                    if sig == last_sig and clean:
                        removed += 1
                        continue
                    last_sig = sig
                keep.append(inst)
            if removed and len(keep) != len(il):
                while il:
                    il.pop()
                il.extend(keep)
    return removed


def _build_program():
    import concourse.bass as bass
    import concourse.tile as tile
    from concourse import bacc, mybir

    _raise_sbuf_cap()

    f16 = mybir.dt.float16
    f32 = mybir.dt.float32
    RELU = mybir.ActivationFunctionType.Relu
    IDENT = mybir.ActivationFunctionType.Identity
    ADD = mybir.AluOpType.add
    MAX = mybir.AluOpType.max

    nc = bacc.Bacc("TRN2", target_bir_lowering=False, debug=False, num_devices=NCORES)

    pack_d = nc.dram_tensor("pack", [128, NPACK, SHARD], f16, kind="ExternalInput").ap()
    # all weights in one host-laid-out fp16 blob, all biases in one fp32 blob
    wb_d = nc.dram_tensor("WB", [128, 2944], f16, kind="ExternalInput").ap()
    bb_d = nc.dram_tensor("BB", [128, 12], f32, kind="ExternalInput").ap()
    out_d = nc.dram_tensor("outT", [10, SHARD], f32, kind="ExternalOutput").ap()

    with tile.TileContext(nc) as tc:
        with (
            tc.tile_pool(name="wpool", bufs=1) as wp,
            tc.tile_pool(name="mk", bufs=2) as mkp,
            tc.tile_pool(name="hr", bufs=1) as hrp,
            tc.tile_pool(name="hm", bufs=1) as hmp,
            tc.tile_pool(name="osb", bufs=2) as outp,
            tc.tile_pool(name="ps", bufs=7, space="PSUM") as psp,
            tc.tile_pool(name="ps678", bufs=1, space="PSUM") as ps678p,
        ):
            wall = wp.tile([128, 2944], f16, tag="wall")
            ball = wp.tile([128, 12], f32, tag="ball")
            # blob column offsets: w1@0(256) w2@256(256) w3@512(1024) w4@1536(1024)
            #   w5@2560(256) w6@2816(64) w789@2880(64: W7 r0-63 c0-31, W8 r64-95
            #   c32-47, W9 r96-111 c48-57)
            WOFF = {1: 0, 2: 256, 3: 512, 4: 1536, 5: 2560, 6: 2816, 789: 2880}
            w789 = wall[:, WOFF[789]:WOFF[789] + 64]
            b15 = ball[:, 0:10]
            b678 = ball[:, 10:11]
            b9 = ball[0:10, 11:12]

            def wslice(l, k, c, N):
                base = WOFF[l] + k * N
                return wall[:, base + c * 128: base + (c + 1) * 128]

            RELUf, IDENTf, ADDf, MAXf = RELU, IDENT, ADD, MAX

            def drain_relu(eng, dst, zsrc, bias_ap):
                if eng == "act":
                    nc.scalar.activation(dst, zsrc, RELUf, bias=bias_ap)
                else:
                    nc.vector.tensor_scalar(dst, zsrc, bias_ap, 0.0, ADDf, MAXf)

            def mask_mul(eng, dst, src, msrc):
                if eng == "gps":
                    nc.gpsimd.tensor_mul(dst, src, msrc)
                else:
                    nc.vector.tensor_mul(dst, src, msrc)

            def sub(ap3, c, t):
                return ap3[:, c, bass.ts(t, MSUB)]

            state = {}

            def emit_ladder_step(b, step):
                st = state[b]
                hm5, m678 = st["hm5"], st["m678"]
                if step == 0:
                    st["hr678"] = hrp.tile([128, 1, BLK], f16, tag="hr678",
                                           name=f"hr678_{b}", bufs=2)
                    st["hm678"] = st["hr678"]
                    st["zh"] = [psp.tile([128, MSUB], f32, tag="ps",
                                         name=f"z678_{b}_{h}") for h in range(NSUB)]
                hr678, hm678, zh = st["hr678"], st["hm678"], st["zh"]
                cfg = [((0, 64), wall[:, WOFF[6]:WOFF[6] + 64], None, None),
                       ((64, 96), w789[0:64, 0:32], (0, 64), (0, 64)),
                       ((96, 112), w789[64:96, 32:48], (64, 96), (64, 96))][step]
                (p0, p1), wap, tile_pos, brange = cfg
                for t in range(NSUB):
                    ts_ = bass.ts(t, MSUB)
                    rhs = (sub(hm5, 0, t) if step == 0 else
                           hm678[brange[0]:brange[1], 0, ts_])
                    if tile_pos is None:
                        nc.tensor.matmul(zh[t][p0:p1, :], wap, rhs,
                                         start=True, stop=True)
                    else:
                        nc.tensor.matmul(zh[t][p0:p1, :], wap, rhs,
                                         start=True, stop=True, tile_position=tile_pos)
                for t in range(NSUB):
                    ts_ = bass.ts(t, MSUB)
                    drain_relu("dve" if t % 2 == 0 else "act",
                               hr678[p0:p1, 0, ts_], zh[t][p0:p1, :],
                               b678[p0:p1, 0:1])
                    mask_mul("dve" if t % 2 == 0 else "gps",
                             hr678[p0:p1, 0, ts_], hr678[p0:p1, 0, ts_],
                             state[b]["m678"][p0:p1, 0, ts_])

            def emit_l9(b):
                st = state[b]
                hm678 = st["hm678"]
                osb = outp.tile([10, BLK], f32, tag="osb", bufs=2, name=f"osb_{b}")
                for t in range(NSUB):
                    z9 = psp.tile([128, MSUB], f32, tag="ps", name=f"z9_{b}_{t}")
                    nc.tensor.matmul(z9[0:10, :], w789[96:112, 48:58],
                                     hm678[96:112, 0, bass.ts(t, MSUB)],
                                     start=True, stop=True, tile_position=(96, 0))
                    nc.scalar.activation(osb[:, bass.ts(t, MSUB)], z9[0:10, :], IDENTf, bias=b9[:, 0:1])
                nc.sync.dma_start(out_d[:, bass.ts(b, BLK)], osb[:])
                del state[b]

            dr_i = [0]

            def pick_drain():
                i = dr_i[0]
                dr_i[0] += 1
                return "act" if (i * 27) % 40 < 27 else "dve"

            for b in range(NBLK):
                bs = bass.ts(b, BLK)
                pkx = mkp.tile([128, 2, BLK], f16, tag="pkx", name=f"pkx_{b}")
                m1 = mkp.tile([128, 1, BLK], f16, tag="m1", name=f"m1_{b}")
                m2 = mkp.tile([128, 2, BLK], f16, tag="m2", name=f"m2_{b}")
                m3 = mkp.tile([128, 4, BLK], f16, tag="m3", name=f"m3_{b}")
                m4 = mkp.tile([128, 2, BLK], f16, tag="m4", name=f"m4_{b}")
                m5 = mkp.tile([128, 1, BLK], f16, tag="m5", name=f"m5_{b}")
                m678 = mkp.tile([128, 1, BLK], f16, tag="m678", name=f"m678_{b}")
                nc.sync.dma_start(pkx[:], pack_d[:, 0:2, bs])
                nc.sync.dma_start(m1[:], pack_d[:, 2:3, bs])
                nc.sync.dma_start(m2[:], pack_d[:, 3:5, bs])
                nc.sync.dma_start(m3[:], pack_d[:, 5:9, bs])
                nc.sync.dma_start(m4[:], pack_d[:, 9:11, bs])
                nc.sync.dma_start(m5[:], pack_d[:, 11:12, bs])
                nc.sync.dma_start(m678[:], pack_d[:, 12:13, bs])
                if b == 0:
                    # weights after the first block's inputs: 2 issues, overlaps pkx
                    nc.sync.dma_start(wall[:], wb_d[:])
                    nc.sync.dma_start(ball[:], bb_d[:])

                layer_cfg = [
                    (2, 1, 128, 1, m1, 0, "hr1", "hm1", "dve"),
                    (1, 2, 256, 2, m2, 1, "hr2", "hm2", "dve"),
                    (2, 3, 512, 4, m3, 3, "hr3", "hm3", "dve"),
                    (4, 4, 256, 2, m4, 7, "hr4", "hm4", "dve"),
                    (2, 5, 128, 1, m5, 9, "hr5", "hm5", "dve"),
                ]
                for li, (Kc, wl, wN, Cc, mt, boff, hrtag, hmtag, mpol) in enumerate(layer_cfg):
                    hr = hrp.tile([128, Cc, BLK], f16, tag=hrtag, name=hrtag + f"_{b}",
                                  bufs=2 if hrtag in ("hr5", "hr3", "hr2") else 1)
                    hin = pkx if li == 0 else prev_hm
                    zs = {}
                    for c in range(Cc):
                        for t in range(NSUB):
                            zs[c, t] = psp.tile([128, MSUB], f32, tag="ps",
                                                name=f"z_{hrtag}_{b}_{c}_{t}")
                    for c in range(Cc):
                        for k in range(Kc):
                            wap = wslice(wl, k, c, wN)
                            for t in range(NSUB):
                                nc.tensor.matmul(zs[c, t][:], wap, sub(hin, k, t),
                                                 start=(k == 0), stop=(k == Kc - 1))
                    for t in range(NSUB):
                        for c in range(Cc):
                            drain_relu(pick_drain(), sub(hr, c, t), zs[c, t][:],
                                       b15[:, boff + c:boff + c + 1])
                    # contiguous per-(chunk, half) mask ops: DVE 2x mode + fine deps.
                    # GPS takes the tail chunks (their consumer k-groups run last,
                    # so the slower engine is covered by earlier k-group matmuls).
                    for c in range(Cc):
                        for p in range(2):
                            hs = bass.ts(p, BLK // 2)
                            mask_mul("dve", hr[:, c, hs], hr[:, c, hs], mt[:, c, hs])
                    prev_hm = hr
                    # software pipelining: previous block's small-layer ladder steps
                    # interleave between this block's big-layer bursts
                    if b > 0 and li >= 1 and li <= 3 and (b - 1) in state:
                        emit_ladder_step(b - 1, li - 1)
                    if b > 0 and li == 4 and (b - 1) in state:
                        emit_l9(b - 1)

                state[b] = {"hm5": prev_hm, "m678": m678}

            # tail: smalls for the last block
            emit_ladder_step(NBLK - 1, 0)
            emit_ladder_step(NBLK - 1, 1)
            emit_ladder_step(NBLK - 1, 2)
            emit_l9(NBLK - 1)

    _dedup_ldweights(nc)
    nc.compile()
    return nc


def _get_program():
    if "nc" not in _PROG:
        _PROG["nc"] = _build_program()
    return _PROG["nc"]


def _host_prep(inputs):
    """Build per-core input maps (numpy only)."""
    x = np.asarray(inputs["x"], dtype=np.float32)
    Ws = [np.asarray(inputs[f"W{i}"], dtype=np.float32) for i in range(1, 10)]
    bs = [np.asarray(inputs[f"b{i}"], dtype=np.float32) for i in range(1, 10)]
    ms = [np.asarray(inputs[f"m{i}"], dtype=np.float32) for i in range(1, 9)]

    # fold dropout scale into next layer's weights; binarize masks
    Wf = [Ws[0]]
    for i in range(1, 9):
        s = float(ms[i - 1].max())
        if s <= 0.0:  # degenerate all-dropped mask; keep weights unscaled
            s = 1.0
        Wf.append(Ws[i] * np.float32(s))

    # weight blob: w1@0 w2@256 w3@512 w4@1536 w5@2560 w6@2816 w789@2880
    WOFF = {1: 0, 2: 256, 3: 512, 4: 1536, 5: 2560, 6: 2816, 789: 2880}
    wb = np.zeros((128, 2944), dtype=np.float16)
    for l in range(1, 7):
        W = Wf[l - 1]
        K, N = W.shape
        for k in range((K + 127) // 128):
            blk = W[k * 128:(k + 1) * 128].astype(np.float16)
            wb[: blk.shape[0], WOFF[l] + k * N: WOFF[l] + k * N + N] = blk
    wb[0:64, 2880:2912] = Wf[6].astype(np.float16)    # W7
    wb[64:96, 2912:2928] = Wf[7].astype(np.float16)   # W8
    wb[96:112, 2928:2938] = Wf[8].astype(np.float16)  # W9
    bb = np.zeros((128, 12), dtype=np.float32)
    bb[:, 0] = bs[0]
    bb[:, 1], bb[:, 2] = bs[1][0:128], bs[1][128:256]
    for c in range(4):
        bb[:, 3 + c] = bs[2][c * 128:(c + 1) * 128]
    bb[:, 7], bb[:, 8] = bs[3][0:128], bs[3][128:256]
    bb[:, 9] = bs[4]
    bb[0:64, 10], bb[64:96, 10], bb[96:112, 10] = bs[5], bs[6], bs[7]
    bb[0:10, 11] = bs[8]
    shared = {"WB": wb, "BB": bb}

    in_maps = []
    for c in range(NCORES):
        sl = slice(c * SHARD, (c + 1) * SHARD)
        pack = np.zeros((128, NPACK, SHARD), dtype=np.float16)
        xT = x[sl].T  # (256, SHARD)
        pack[:, 0, :] = xT[0:128]
        pack[:, 1, :] = xT[128:256]
        mT = [None] + [(ms[i][sl] != 0).T.astype(np.float16) for i in range(8)]  # 1-indexed
        pack[:, 2, :] = mT[1]
        pack[:, 3, :], pack[:, 4, :] = mT[2][0:128], mT[2][128:256]
        for k in range(4):
            pack[:, 5 + k, :] = mT[3][k * 128:(k + 1) * 128]
        pack[:, 9, :], pack[:, 10, :] = mT[4][0:128], mT[4][128:256]
        pack[:, 11, :] = mT[5]
        pack[0:64, 12, :] = mT[6]
        pack[64:96, 12, :] = mT[7]
        pack[96:112, 12, :] = mT[8]
        in_maps.append({"pack": pack, **shared})
    return in_maps


def kernel(**inputs) -> np.ndarray:
    from concourse.bass_utils import run_bass_kernel_spmd

    nc = _get_program()
    in_maps = _host_prep(inputs)
    res = run_bass_kernel_spmd(nc, in_maps, list(range(NCORES)))
    out = np.empty((BATCH, DIMS[-1]), dtype=np.float32)
    for c in range(NCORES):
        out[c * SHARD:(c + 1) * SHARD, :] = res.results[c]["outT"].T
    return out


# revision 32
# speedup vs baseline: 1.1189x; 1.1189x over previous
"""Trainium2 Bass kernel for the 9-layer dense MLP (dropout-mask training forward).

Strategy (pure data parallel, 8 cores, 8192 batch rows each):
  - Activations kept transposed on-chip: features on partitions, batch cols on free dim.
    Each layer computes zT = W^T @ hT via nc.tensor.matmul(out, lhsT=W, rhs=hT).
  - fp16 weights/activations/masks (fp32 PSUM accumulation), fp32 biases + output.
  - Dropout masks binarized on host ({0,1} fp16); the 1/keep scale is folded into the
    next layer's weights.
  - Layer-major processing within blocks of 2048 batch columns (4 PSUM sub-tiles of
    512): per (c,k) weight tile, 4 consecutive matmuls share one LDWEIGHTS and
    pipeline back-to-back on the PE; PSUM drained per 512-chunk with fused bias+relu
    alternating ScalarE/VectorE; mask multiply per (layer, sub-tile) on DVE/GpSimd.
  - Layers 6/7/8 partition-packed (partition offsets 0/64/96 via matmul tile_position)
    sharing 2-bank PSUM tiles per 1024-column half and one packed mask chunk.
"""

import sys

sys.path.insert(0, "/opt/trn_rl_repo")

import numpy as np

DIMS = [256, 128, 256, 512, 256, 128, 64, 32, 16, 10]
NCORES = 8
BATCH = 65536
SHARD = BATCH // NCORES  # 8192
MSUB = 512               # PSUM sub-tile columns
BLK = 2048               # block columns
NBLK = SHARD // BLK      # 4
NSUB = BLK // MSUB       # 4

# pack chunk layout (each chunk = 128 partitions x 8192 cols, fp16):
#   0,1: xT        2: m1        3,4: m2      5-8: m3
#   9,10: m4       11: m5       12: m6/m7/m8 partition-packed at rows 0/64/96
NPACK = 13

_PROG = {}


def _raise_sbuf_cap():
    # tile_utils.max_sbuf_usage is a stale 192KB constant; cayman has 208KB usable.
    import concourse.tile_utils as tu

    if getattr(tu, "max_sbuf_usage", 0) < 206 * 1024:
        tu.max_sbuf_usage = 206 * 1024


def _dedup_ldweights(nc):
    """Remove back-to-back redundant LDWEIGHTS (same stationary operand) so
    consecutive same-weight matmuls pipeline on the PE. Only drops LDW
    instructions that carry no semaphore waits/updates."""
    removed = 0
    for fn in nc.m.functions:
        for blk in fn.blocks:
            il = blk.instructions
            keep, last_sig = [], None
            for inst in il:
                nm = type(inst).__name__
                if nm == "InstLdweights":
                    sig = (str(inst.ins[0]), str(inst.is_transpose), str(inst.perf_mode),
                           str(getattr(inst, "tile_position", None)))
                    si = inst.sync_info
                    clean = si is None or (not si.on_wait and not si.on_update)
                    if sig == last_sig and clean:
                        removed += 1
                        continue
                    last_sig = sig
                keep.append(inst)
            if removed and len(keep) != len(il):
                while il:
                    il.pop()
                il.extend(keep)
    return removed


def _build_program():
    import concourse.bass as bass
    import concourse.tile as tile
    from concourse import bacc, mybir

    _raise_sbuf_cap()

    f16 = mybir.dt.float16
    f32 = mybir.dt.float32
    RELU = mybir.ActivationFunctionType.Relu
    IDENT = mybir.ActivationFunctionType.Identity
    ADD = mybir.AluOpType.add
    MAX = mybir.AluOpType.max

    nc = bacc.Bacc("TRN2", target_bir_lowering=False, debug=False, num_devices=NCORES)

    pack_d = nc.dram_tensor("pack", [128, NPACK, SHARD], f16, kind="ExternalInput").ap()
    # all weights in one host-laid-out fp16 blob, all biases in one fp32 blob
    wb_d = nc.dram_tensor("WB", [128, 2944], f16, kind="ExternalInput").ap()
    bb_d = nc.dram_tensor("BB", [128, 12], f32, kind="ExternalInput").ap()
    out_d = nc.dram_tensor("outT", [10, SHARD], f32, kind="ExternalOutput").ap()

    with tile.TileContext(nc) as tc:
        with (
            tc.tile_pool(name="wpool", bufs=1) as wp,
            tc.tile_pool(name="mk", bufs=2) as mkp,
            tc.tile_pool(name="hr", bufs=1) as hrp,
            tc.tile_pool(name="hm", bufs=1) as hmp,
            tc.tile_pool(name="osb", bufs=2) as outp,
            tc.tile_pool(name="ps", bufs=7, space="PSUM") as psp,
            tc.tile_pool(name="ps678", bufs=1, space="PSUM") as ps678p,
        ):
            wall = wp.tile([128, 2944], f16, tag="wall")
            ball = wp.tile([128, 12], f32, tag="ball")
            # blob column offsets: w1@0(256) w2@256(256) w3@512(1024) w4@1536(1024)
            #   w5@2560(256) w6@2816(64) w789@2880(64: W7 r0-63 c0-31, W8 r64-95
            #   c32-47, W9 r96-111 c48-57)
            WOFF = {1: 0, 2: 256, 3: 512, 4: 1536, 5: 2560, 6: 2816, 789: 2880}
            w789 = wall[:, WOFF[789]:WOFF[789] + 64]
            b15 = ball[:, 0:10]
            b678 = ball[:, 10:11]
            b9 = ball[0:10, 11:12]

            def wslice(l, k, c, N):
                base = WOFF[l] + k * N
                return wall[:, base + c * 128: base + (c + 1) * 128]

            RELUf, IDENTf, ADDf, MAXf = RELU, IDENT, ADD, MAX

            def drain_relu(eng, dst, zsrc, bias_ap):
                if eng == "act":
                    nc.scalar.activation(dst, zsrc, RELUf, bias=bias_ap)
                else:
                    nc.vector.tensor_scalar(dst, zsrc, bias_ap, 0.0, ADDf, MAXf)

            def mask_mul(eng, dst, src, msrc):
                if eng == "gps":
                    nc.gpsimd.tensor_mul(dst, src, msrc)
                else:
                    nc.vector.tensor_mul(dst, src, msrc)

            def sub(ap3, c, t):
                return ap3[:, c, bass.ts(t, MSUB)]

            state = {}

            def emit_ladder_step(b, step):
                st = state[b]
                hm5, m678 = st["hm5"], st["m678"]
                if step == 0:
                    st["hr678"] = hrp.tile([128, 1, BLK], f16, tag="hr678",
                                           name=f"hr678_{b}", bufs=2)
                    st["hm678"] = st["hr678"]
                    st["zh"] = [psp.tile([128, MSUB], f32, tag="ps",
                                         name=f"z678_{b}_{h}") for h in range(NSUB)]
                hr678, hm678, zh = st["hr678"], st["hm678"], st["zh"]
                cfg = [((0, 64), wall[:, WOFF[6]:WOFF[6] + 64], None, None),
                       ((64, 96), w789[0:64, 0:32], (0, 64), (0, 64)),
                       ((96, 112), w789[64:96, 32:48], (64, 96), (64, 96))][step]
                (p0, p1), wap, tile_pos, brange = cfg
                for t in range(NSUB):
                    ts_ = bass.ts(t, MSUB)
                    rhs = (sub(hm5, 0, t) if step == 0 else
                           hm678[brange[0]:brange[1], 0, ts_])
                    if tile_pos is None:
                        nc.tensor.matmul(zh[t][p0:p1, :], wap, rhs,
                                         start=True, stop=True)
                    else:
                        nc.tensor.matmul(zh[t][p0:p1, :], wap, rhs,
                                         start=True, stop=True, tile_position=tile_pos)
                for t in range(NSUB):
                    ts_ = bass.ts(t, MSUB)
                    drain_relu("dve" if t % 2 == 0 else "act",
                               hr678[p0:p1, 0, ts_], zh[t][p0:p1, :],
                               b678[p0:p1, 0:1])
                    mask_mul("dve" if t % 2 == 0 else "gps",
                             hr678[p0:p1, 0, ts_], hr678[p0:p1, 0, ts_],
                             state[b]["m678"][p0:p1, 0, ts_])

            def emit_l9(b):
                st = state[b]
                hm678 = st["hm678"]
                osb = outp.tile([10, BLK], f32, tag="osb", bufs=2, name=f"osb_{b}")
                for t in range(NSUB):
                    z9 = psp.tile([128, MSUB], f32, tag="ps", name=f"z9_{b}_{t}")
                    nc.tensor.matmul(z9[0:10, :], w789[96:112, 48:58],
                                     hm678[96:112, 0, bass.ts(t, MSUB)],
                                     start=True, stop=True, tile_position=(96, 0))
                    nc.scalar.activation(osb[:, bass.ts(t, MSUB)], z9[0:10, :], IDENTf, bias=b9[:, 0:1])
                nc.sync.dma_start(out_d[:, bass.ts(b, BLK)], osb[:])
                del state[b]

            dr_i = [0]

            def pick_drain():
                i = dr_i[0]
                dr_i[0] += 1
                return "act" if (i * 27) % 40 < 27 else "dve"

            nc.sync.dma_start(wall[:], wb_d[:])
            nc.sync.dma_start(ball[:], bb_d[:])

            for b in range(NBLK):
                bs = bass.ts(b, BLK)
                pkx = mkp.tile([128, 2, BLK], f16, tag="pkx", name=f"pkx_{b}")
                m1 = mkp.tile([128, 1, BLK], f16, tag="m1", name=f"m1_{b}")
                m2 = mkp.tile([128, 2, BLK], f16, tag="m2", name=f"m2_{b}")
                m3 = mkp.tile([128, 4, BLK], f16, tag="m3", name=f"m3_{b}")
                m4 = mkp.tile([128, 2, BLK], f16, tag="m4", name=f"m4_{b}")
                m5 = mkp.tile([128, 1, BLK], f16, tag="m5", name=f"m5_{b}")
                m678 = mkp.tile([128, 1, BLK], f16, tag="m678", name=f"m678_{b}")
                nc.sync.dma_start(pkx[:], pack_d[:, 0:2, bs])
                nc.sync.dma_start(m1[:], pack_d[:, 2:3, bs])
                nc.sync.dma_start(m2[:], pack_d[:, 3:5, bs])
                nc.sync.dma_start(m3[:], pack_d[:, 5:9, bs])
                nc.sync.dma_start(m4[:], pack_d[:, 9:11, bs])
                nc.sync.dma_start(m5[:], pack_d[:, 11:12, bs])
                nc.sync.dma_start(m678[:], pack_d[:, 12:13, bs])

                layer_cfg = [
                    (2, 1, 128, 1, m1, 0, "hr1", "hm1", "dve"),
                    (1, 2, 256, 2, m2, 1, "hr2", "hm2", "dve"),
                    (2, 3, 512, 4, m3, 3, "hr3", "hm3", "dve"),
                    (4, 4, 256, 2, m4, 7, "hr4", "hm4", "dve"),
                    (2, 5, 128, 1, m5, 9, "hr5", "hm5", "dve"),
                ]
                for li, (Kc, wl, wN, Cc, mt, boff, hrtag, hmtag, mpol) in enumerate(layer_cfg):
                    hr = hrp.tile([128, Cc, BLK], f16, tag=hrtag, name=hrtag + f"_{b}",
                                  bufs=2 if hrtag in ("hr5", "hr3", "hr2") else 1)
                    hin = pkx if li == 0 else prev_hm
                    zs = {}
                    for c in range(Cc):
                        for t in range(NSUB):
                            zs[c, t] = psp.tile([128, MSUB], f32, tag="ps",
                                                name=f"z_{hrtag}_{b}_{c}_{t}")
                    for c in range(Cc):
                        for k in range(Kc):
                            wap = wslice(wl, k, c, wN)
                            for t in range(NSUB):
                                nc.tensor.matmul(zs[c, t][:], wap, sub(hin, k, t),
                                                 start=(k == 0), stop=(k == Kc - 1))
                    for t in range(NSUB):
                        for c in range(Cc):
                            drain_relu(pick_drain(), sub(hr, c, t), zs[c, t][:],
                                       b15[:, boff + c:boff + c + 1])
                    # contiguous per-(chunk, half) mask ops: DVE 2x mode + fine deps.
                    # GPS takes the tail chunks (their consumer k-groups run last,
                    # so the slower engine is covered by earlier k-group matmuls).
                    for c in range(Cc):
                        for p in range(2):
                            hs = bass.ts(p, BLK // 2)
                            mask_mul("dve", hr[:, c, hs], hr[:, c, hs], mt[:, c, hs])
                    prev_hm = hr
                    # software pipelining: previous block's small-layer ladder steps
                    # interleave between this block's big-layer bursts
                    if b > 0 and li >= 1 and li <= 3 and (b - 1) in state:
                        emit_ladder_step(b - 1, li - 1)
                    if b > 0 and li == 4 and (b - 1) in state:
                        emit_l9(b - 1)

                state[b] = {"hm5": prev_hm, "m678": m678}

            # tail: smalls for the last block
            emit_ladder_step(NBLK - 1, 0)
            emit_ladder_step(NBLK - 1, 1)
            emit_ladder_step(NBLK - 1, 2)
            emit_l9(NBLK - 1)

    _dedup_ldweights(nc)
    nc.compile()
    return nc


def _get_program():
    if "nc" not in _PROG:
        _PROG["nc"] = _build_program()
    return _PROG["nc"]


def _host_prep(inputs):
    """Build per-core input maps (numpy only)."""
    x = np.asarray(inputs["x"], dtype=np.float32)
    Ws = [np.asarray(inputs[f"W{i}"], dtype=np.float32) for i in range(1, 10)]
    bs = [np.asarray(inputs[f"b{i}"], dtype=np.float32) for i in range(1, 10)]
    ms = [np.asarray(inputs[f"m{i}"], dtype=np.float32) for i in range(1, 9)]

    # fold dropout scale into next layer's weights; binarize masks
    Wf = [Ws[0]]
    for i in range(1, 9):
        s = float(ms[i - 1].max())
        if s <= 0.0:  # degenerate all-dropped mask; keep weights unscaled
            s = 1.0
        Wf.append(Ws[i] * np.float32(s))

    # weight blob: w1@0 w2@256 w3@512 w4@1536 w5@2560 w6@2816 w789@2880
    WOFF = {1: 0, 2: 256, 3: 512, 4: 1536, 5: 2560, 6: 2816, 789: 2880}
    wb = np.zeros((128, 2944), dtype=np.float16)
    for l in range(1, 7):
        W = Wf[l - 1]
        K, N = W.shape
        for k in range((K + 127) // 128):
            blk = W[k * 128:(k + 1) * 128].astype(np.float16)
            wb[: blk.shape[0], WOFF[l] + k * N: WOFF[l] + k * N + N] = blk
    wb[0:64, 2880:2912] = Wf[6].astype(np.float16)    # W7
    wb[64:96, 2912:2928] = Wf[7].astype(np.float16)   # W8
    wb[96:112, 2928:2938] = Wf[8].astype(np.float16)  # W9
    bb = np.zeros((128, 12), dtype=np.float32)
    bb[:, 0] = bs[0]
    bb[:, 1], bb[:, 2] = bs[1][0:128], bs[1][128:256]
    for c in range(4):
        bb[:, 3 + c] = bs[2][c * 128:(c + 1) * 128]
    bb[:, 7], bb[:, 8] = bs[3][0:128], bs[3][128:256]
    bb[:, 9] = bs[4]
    bb[0:64, 10], bb[64:96, 10], bb[96:112, 10] = bs[5], bs[6], bs[7]
    bb[0:10, 11] = bs[8]
    shared = {"WB": wb, "BB": bb}

    in_maps = []
    for c in range(NCORES):
        sl = slice(c * SHARD, (c + 1) * SHARD)
        pack = np.zeros((128, NPACK, SHARD), dtype=np.float16)
        xT = x[sl].T  # (256, SHARD)
        pack[:, 0, :] = xT[0:128]
        pack[:, 1, :] = xT[128:256]
        mT = [None] + [(ms[i][sl] != 0).T.astype(np.float16) for i in range(8)]  # 1-indexed
        pack[:, 2, :] = mT[1]
        pack[:, 3, :], pack[:, 4, :] = mT[2][0:128], mT[2][128:256]
        for k in range(4):
            pack[:, 5 + k, :] = mT[3][k * 128:(k + 1) * 128]
        pack[:, 9, :], pack[:, 10, :] = mT[4][0:128], mT[4][128:256]
        pack[:, 11, :] = mT[5]
        pack[0:64, 12, :] = mT[6]
        pack[64:96, 12, :] = mT[7]
        pack[96:112, 12, :] = mT[8]
        in_maps.append({"pack": pack, **shared})
    return in_maps


def kernel(**inputs) -> np.ndarray:
    from concourse.bass_utils import run_bass_kernel_spmd

    nc = _get_program()
    in_maps = _host_prep(inputs)
    res = run_bass_kernel_spmd(nc, in_maps, list(range(NCORES)))
    out = np.empty((BATCH, DIMS[-1]), dtype=np.float32)
    for c in range(NCORES):
        out[c * SHARD:(c + 1) * SHARD, :] = res.results[c]["outT"].T
    return out
